# revision 3
# baseline (speedup 1.0000x reference)
# Bass/Trainium2 kernel for nn_MENet (scatter_memory).
#
# Strategy: pure data parallel over batch (512 -> 64 per core, 8 cores).
# Host pre-folds BN scales into weights, fuses mlp_w1 @ memory_w.T (so the
# [B,64,32] memory read-out is never materialized), permutes fc1 weight
# columns to match the on-chip maxpool layout, and packs all weights into a
# single [128, NW] tensor already in SBUF layout.
#
# On chip (per core):
#   - l3_points/x2_points row maxes: streamed [128, 2, 8, C] tiles (4/8KB
#     contiguous per partition), one segmented DVE reduce per 2 batches.
#   - memory addressing: stacked [values; squares] SBUF tile so ONE K=128
#     matmul per 128 (b,n) rows yields both logits (N=16) and sum-of-squares
#     (col 16); softmax on DVE/ACT; attention rows transposed via PE into the
#     fused MLP, ReLU+bias on ACT (per-partition bias), segmented max into
#     head-input layout.
#   - heads: small PE matmuls (K-chunks of 128) + ACT bias/ReLU; log_softmax
#     via PE transpose + Exp-with-accum + Ln.
import os
from contextlib import ExitStack

import numpy as np

import concourse.bacc as bacc
import concourse.bass as bass
import concourse.tile as tile
from concourse import mybir
from concourse.bass_utils import run_bass_kernel_spmd

F32 = mybir.dt.float32
AF = mybir.ActivationFunctionType
ALU = mybir.AluOpType
AX = mybir.AxisListType

P = 128
NCORES = 8
B = 512
BL = B // NCORES          # 64 batches per core
NM = 32                   # n points per memory block
CM = 64                   # memory channel dim
ROWS = BL * NM            # 2048 rows per core per branch
NGROUP = ROWS // 512      # 4 groups of 512 rows (16 batches each)
EPS_BN = 1e-5


# ----------------------------------------------------------------------------
# host-side weight folding + packing
# ----------------------------------------------------------------------------
class _Pack:
    def __init__(self):
        self.parts = []
        self.off = {}
        self.pos = 0

    def add(self, name, arr):
        arr = np.asarray(arr, np.float32)
        assert arr.ndim == 2 and arr.shape[0] <= P
        buf = np.zeros((P, arr.shape[1]), np.float32)
        buf[: arr.shape[0]] = arr
        self.off[name] = (self.pos, arr.shape[1])
        self.pos += arr.shape[1]
        self.parts.append(buf)

    def finish(self):
        return np.ascontiguousarray(np.concatenate(self.parts, axis=1))


def _perm_pts(npref, npts):
    # device x-vector position npref + j*128 + q  <-  original point 8q + j
    d = np.arange(npts)
    src = npref + 8 * (d % 128) + (d // 128)
    return np.concatenate([np.arange(npref), src])


def _kpack(w_t):  # [K, M] -> [128, nk, M] flattened to [128, nk*M]
    K, M = w_t.shape
    nk = K // P
    return np.ascontiguousarray(
        np.transpose(w_t.reshape(nk, P, M), (1, 0, 2)).reshape(P, nk * M)
    )


def _fold_and_pack(f):
    s = lambda g: g / np.sqrt(1.0 + EPS_BN)
    mw = f["memory_w"]                                    # [16, 64]
    mn = mw / np.maximum(np.linalg.norm(mw, axis=1, keepdims=True), 1e-12)

    pk = _Pack()
    pk.add("ident", np.eye(P, dtype=np.float32))

    rhs2 = np.zeros((P, 17), np.float32)
    rhs2[0:CM, 0:16] = mn.T                               # logits part
    rhs2[CM:2 * CM, 16] = 1.0                             # sum-of-squares part
    pk.add("rhs2", rhs2)

    # branch mlps (conv 1x1): fold BN scale into weights, fuse layer1 with
    # memory_w read-out:  y1[o, row] = sum_s W1e[o, s] * a[row, s]
    for bi, (w1, g1, b1, w2, g2, b2) in enumerate(
        [
            (f["mlp1_w1"], f["mlp1_g1"], f["mlp1_b1"], f["mlp1_w2"], f["mlp1_g2"], f["mlp1_b2"]),
            (f["mlp2_w1"], f["mlp2_g1"], f["mlp2_b1"], f["mlp2_w2"], f["mlp2_g2"], f["mlp2_b2"]),
        ]
    ):
        w1e = (s(g1)[:, None] * w1) @ mw.T                # [M1, 16]
        w2f = s(g2)[:, None] * w2                         # [M2, M1]
        M1, M2 = w2f.shape[1], w2f.shape[0]
        pk.add(f"w1eT_b{bi + 1}", w1e.T)                  # [16, M1]
        pk.add(f"b1_b{bi + 1}", b1.reshape(M1 // P, P).T) # [128, M1/128]
        pk.add(f"w2T_b{bi + 1}", _kpack(w2f.T))           # [128, (M1/128)*M2]
        pk.add(f"b2_b{bi + 1}", b2.reshape(M2 // P, P).T)

    # heads: fold BN into fc1/fc2, permute fc1 cols for the maxpool layout
    for hi, (w1, b1, g1, bb1, w2, b2, g2, bb2, w3, b3, npref) in enumerate(
        [
            (f["fc1_w"], f["fc1_b"], f["bn1_g"], f["bn1_b"], f["fc2_w"], f["fc2_b"],
             f["bn2_g"], f["bn2_b"], f["fc3_w"], f["fc3_b"], 256),
            (f["fc1_2_w"], f["fc1_2_b"], f["bn1_2_g"], f["bn1_2_b"], f["fc2_2_w"],
             f["fc2_2_b"], f["bn2_2_g"], f["bn2_2_b"], f["fc3_2_w"], f["fc3_2_b"], 512),
        ]
    ):
        s1, s2 = s(g1), s(g2)
        w1f = (s1[:, None] * w1)[:, _perm_pts(npref, 1024)]   # [512, npref+1024]
        b1f = s1 * b1 + bb1
        w2f = s2[:, None] * w2                                # [256, 512]
        b2f = s2 * b2 + bb2
        pk.add(f"fw1_h{hi + 1}", _kpack(w1f.T))               # [128, nk1*512]
        pk.add(f"fb1_h{hi + 1}", b1f.reshape(4, P).T)
        pk.add(f"fw2_h{hi + 1}", _kpack(w2f.T))               # [128, 4*256]
        pk.add(f"fb2_h{hi + 1}", b2f.reshape(2, P).T)
        pk.add(f"fw3_h{hi + 1}", _kpack(w3.T))                # [128, 2*40]
        pk.add(f"fb3_h{hi + 1}", b3.reshape(40, 1))

    return pk.finish(), pk.off


# ----------------------------------------------------------------------------
# device program
# ----------------------------------------------------------------------------
def _build(off, NW):
    nc = bacc.Bacc("TRN2", target_bir_lowering=False, debug=False)
    l3d = nc.dram_tensor("l3", [BL, 1024, 128], F32, kind="ExternalInput").ap()
    x2d = nc.dram_tensor("x2", [BL, 1024, 256], F32, kind="ExternalInput").ap()
    mf1d = nc.dram_tensor("mf1", [CM, ROWS], F32, kind="ExternalInput").ap()
    mf2d = nc.dram_tensor("mf2", [CM, ROWS], F32, kind="ExternalInput").ap()
    wpd = nc.dram_tensor("wp", [P, NW], F32, kind="ExternalInput").ap()
    o1d = nc.dram_tensor("out1", [BL, 40], F32, kind="ExternalOutput").ap()
    o2d = nc.dram_tensor("out2", [BL, 40], F32, kind="ExternalOutput").ap()

    with tile.TileContext(nc) as tc, ExitStack() as ctx:
        pp = ctx.enter_context(tc.tile_pool(name="persist", bufs=1))
        wsb = pp.tile([P, NW], F32, name="wsb")
        nc.gpsimd.dma_start(wsb[:], wpd)

        def W(name):
            o, w = off[name]
            return wsb[:, o : o + w]

        eps = pp.tile([P, 1], F32, name="eps")
        nc.vector.memset(eps[:], 1e-24)

        xt_l3 = pp.tile([P, 8, BL], F32, name="xt_l3")   # l3 maxes
        xs = pp.tile([P, 8, BL], F32, name="xs")         # l3max + x2max
        xm1 = pp.tile([P, 2, BL], F32, name="xm1")       # branch1 mlp max
        xm2 = pp.tile([P, 4, BL], F32, name="xm2")       # branch2 mlp max

        # ------------------------------------------------------------------
        # memory-addressing branches (tiny; overlaps the big DMA stream)
        # ------------------------------------------------------------------
        with ExitStack() as bctx:
            brp = bctx.enter_context(tc.tile_pool(name="brp", bufs=2, space="PSUM"))
            brs = bctx.enter_context(tc.tile_pool(name="brs", bufs=3))
            for bi, (mfd, M1, M2, xm) in enumerate(
                [(mf1d, 128, 256, xm1), (mf2d, 256, 512, xm2)]
            ):
                S = pp.tile([P, ROWS], F32, name=f"S{bi}")
                nc.gpsimd.dma_start(S[0:CM, :], mfd)
                # squares on partitions 64..127 so one K=128 matmul gives
                # logits and sum-of-squares together
                nc.sync.dma_start(S[CM : 2 * CM, :], S[0:CM, :])
                nc.scalar.square(S[CM : 2 * CM, :], S[CM : 2 * CM, :])

                for g in range(NGROUP):
                    aTp = brp.tile([16, 512], F32, name="aTp", tag="aTp")
                    for chn in range(4):
                        i = g * 4 + chn
                        lss = brp.tile([P, 17], F32, name="lss", tag="lss")
                        nc.tensor.matmul(
                            lss[:],
                            lhsT=S[:, i * P : (i + 1) * P],
                            rhs=W("rhs2"),
                            start=True,
                            stop=True,
                        )
                        r = brs.tile([P, 1], F32, name="rr", tag="rr")
                        nc.scalar.activation(r[:], lss[:, 16:17], AF.Sqrt, bias=eps[:])
                        rinv = brs.tile([P, 1], F32, name="rinv", tag="rinv")
                        nc.vector.reciprocal(rinv[:], r[:])
                        z = brs.tile([P, 16], F32, name="zz", tag="zz")
                        nc.vector.tensor_scalar(z[:], lss[:, 0:16], rinv[:], None, ALU.mult)
                        nm = brs.tile([P, 1], F32, name="nm", tag="nm")
                        nc.vector.tensor_reduce(nm[:], z[:], axis=AX.X, op=ALU.max, negate=True)
                        e = brs.tile([P, 16], F32, name="ee", tag="ee")
                        se = brs.tile([P, 1], F32, name="se", tag="se")
                        nc.scalar.activation(e[:], z[:], AF.Exp, bias=nm[:], accum_out=se[:])
                        rs = brs.tile([P, 1], F32, name="rs", tag="rs")
                        nc.vector.reciprocal(rs[:], se[:])
                        a = brs.tile([P, 16], F32, name="aa", tag="aa")
                        nc.vector.tensor_scalar(a[:], e[:], rs[:], None, ALU.mult)
                        nc.tensor.transpose(aTp[:, chn * P : (chn + 1) * P], a[:], W("ident"))
                    aT = brs.tile([16, 512], F32, name="aT", tag="aT")
                    nc.scalar.copy(aT[:], aTp[:])

                    y1 = brs.tile([P, M1 // P, 512], F32, name="y1", tag="y1")
                    for mj in range(M1 // P):
                        y1p = brp.tile([P, 512], F32, name="y1p", tag="y1p")
                        nc.tensor.matmul(
                            y1p[:],
                            lhsT=W(f"w1eT_b{bi + 1}")[0:16, mj * P : (mj + 1) * P],
                            rhs=aT[:],
                            start=True,
                            stop=True,
                        )
                        nc.scalar.activation(
                            y1[:, mj, :], y1p[:], AF.Relu,
                            bias=W(f"b1_b{bi + 1}")[:, mj : mj + 1],
                        )
                    for mj2 in range(M2 // P):
                        y2p = brp.tile([P, 512], F32, name="y2p", tag="y2p")
                        for kc in range(M1 // P):
                            nc.tensor.matmul(
                                y2p[:],
                                lhsT=W(f"w2T_b{bi + 1}")[:, kc * M2 + mj2 * P : kc * M2 + (mj2 + 1) * P],
                                rhs=y1[:, kc, :],
                                start=(kc == 0),
                                stop=(kc == M1 // P - 1),
                            )
                        y2 = brs.tile([P, 512], F32, name="y2", tag="y2")
                        nc.scalar.activation(
                            y2[:], y2p[:], AF.Relu,
                            bias=W(f"b2_b{bi + 1}")[:, mj2 : mj2 + 1],
                        )
                        nc.vector.tensor_reduce(
                            xm[:, mj2, g * 16 : (g + 1) * 16],
                            y2.rearrange("p (b n) -> p b n", n=NM),
                            axis=AX.X,
                            op=ALU.max,
                        )

        # ------------------------------------------------------------------
        # big maxpools: l3_points then x2_points, 2 batches per DMA
        # ------------------------------------------------------------------
        with ExitStack() as gctx:
            lp = gctx.enter_context(tc.tile_pool(name="lp", bufs=3))
            xp = gctx.enter_context(tc.tile_pool(name="xp", bufs=3))
            tp = gctx.enter_context(tc.tile_pool(name="tp", bufs=4))
            for bp in range(BL // 2):
                t = lp.tile([P, 2, 8, 128], F32, name="l3t", tag="l3t")
                nc.sync.dma_start(
                    t[:], l3d[2 * bp : 2 * bp + 2].rearrange("b (q j) c -> q b j c", j=8)
                )
                nc.vector.tensor_reduce(
                    xt_l3[:, :, 2 * bp : 2 * bp + 2].rearrange("p j b -> p b j"),
                    t[:],
                    axis=AX.X,
                    op=ALU.max,
                )
            for bp in range(BL // 2):
                t = xp.tile([P, 2, 8, 256], F32, name="x2t", tag="x2t")
                nc.sync.dma_start(
                    t[:], x2d[2 * bp : 2 * bp + 2].rearrange("b (q j) c -> q b j c", j=8)
                )
                tm = tp.tile([P, 2, 8], F32, name="tm", tag="tm")
                nc.vector.tensor_reduce(tm[:], t[:], axis=AX.X, op=ALU.max)
                nc.vector.tensor_tensor(
                    xs[:, :, 2 * bp : 2 * bp + 2].rearrange("p j b -> p b j"),
                    tm[:],
                    xt_l3[:, :, 2 * bp : 2 * bp + 2].rearrange("p j b -> p b j"),
                    ALU.add,
                )

        # ------------------------------------------------------------------
        # heads
        # ------------------------------------------------------------------
        with ExitStack() as hctx:
            hp = hctx.enter_context(tc.tile_pool(name="hp", bufs=2, space="PSUM"))
            hs = hctx.enter_context(tc.tile_pool(name="hs", bufs=2))
            for hi, (xmh, pts, npref, odram) in enumerate(
                [(xm1, xt_l3, 2, o1d), (xm2, xs, 4, o2d)]
            ):
                rhs_chunks = [xmh[:, j, :] for j in range(npref)] + [
                    pts[:, j, :] for j in range(8)
                ]
                nk1 = len(rhs_chunks)
                h1 = hs.tile([P, 4, BL], F32, name=f"h1_{hi}", tag="h1")
                for mj in range(4):
                    pp1 = hp.tile([P, BL], F32, name="pp1", tag="pp1")
                    for kc in range(nk1):
                        nc.tensor.matmul(
                            pp1[:],
                            lhsT=W(f"fw1_h{hi + 1}")[:, kc * 512 + mj * P : kc * 512 + (mj + 1) * P],
                            rhs=rhs_chunks[kc],
                            start=(kc == 0),
                            stop=(kc == nk1 - 1),
                        )
                    nc.scalar.activation(
                        h1[:, mj, :], pp1[:], AF.Relu,
                        bias=W(f"fb1_h{hi + 1}")[:, mj : mj + 1],
                    )
                h2 = hs.tile([P, 2, BL], F32, name=f"h2_{hi}", tag="h2")
                for mj in range(2):
                    pp2 = hp.tile([P, BL], F32, name="pp2", tag="pp1")
                    for kc in range(4):
                        nc.tensor.matmul(
                            pp2[:],
                            lhsT=W(f"fw2_h{hi + 1}")[:, kc * 256 + mj * P : kc * 256 + (mj + 1) * P],
                            rhs=h1[:, kc, :],
                            start=(kc == 0),
                            stop=(kc == 3),
                        )
                    nc.scalar.activation(
                        h2[:, mj, :], pp2[:], AF.Relu,
                        bias=W(f"fb2_h{hi + 1}")[:, mj : mj + 1],
                    )
                pp3 = hp.tile([40, BL], F32, name="pp3", tag="pp3")
                for kc in range(2):
                    nc.tensor.matmul(
                        pp3[:],
                        lhsT=W(f"fw3_h{hi + 1}")[:, kc * 40 : (kc + 1) * 40],
                        rhs=h2[:, kc, :],
                        start=(kc == 0),
                        stop=(kc == 1),
                    )
                f3 = hs.tile([40, BL], F32, name=f"f3_{hi}", tag="f3")
                nc.scalar.activation(
                    f3[:], pp3[:], AF.Identity, bias=W(f"fb3_h{hi + 1}")[0:40, 0:1]
                )
                # log_softmax over the 40 classes: transpose to [B, 40]
                zp = hp.tile([BL, 40], F32, name="zp", tag="zp")
                nc.tensor.transpose(zp[:], f3[:], W("ident")[0:40, 0:40])
                z = hs.tile([BL, 40], F32, name=f"z_{hi}", tag="z")
                nc.scalar.copy(z[:], zp[:])
                nm = hs.tile([BL, 1], F32, name="hnm", tag="hnm")
                nc.vector.tensor_reduce(nm[:], z[:], axis=AX.X, op=ALU.max, negate=True)
                e = hs.tile([BL, 40], F32, name="he", tag="he")
                se = hs.tile([BL, 1], F32, name="hse", tag="hse")
                nc.scalar.activation(e[:], z[:], AF.Exp, bias=nm[:], accum_out=se[:])
                lse = hs.tile([BL, 1], F32, name="lse", tag="lse")
                nc.scalar.activation(lse[:], se[:], AF.Ln)
                oo = hs.tile([BL, 40], F32, name=f"oo_{hi}", tag="oo")
                nc.vector.tensor_scalar(oo[:], z[:], nm[:], lse[:], ALU.add, ALU.subtract)
                nc.sync.dma_start(odram, oo[:])

    nc.compile()
    return nc


# ----------------------------------------------------------------------------
# entry point
# ----------------------------------------------------------------------------
_CACHE = {}


def _prep(inputs):
    f = {k: np.ascontiguousarray(np.asarray(v), dtype=np.float32) for k, v in inputs.items()}
    wp, off = _fold_and_pack(f)
    if "nc" not in _CACHE:
        _CACHE["nc"] = _build(off, wp.shape[1])
    in_maps = []
    for c in range(NCORES):
        sl = slice(c * BL, (c + 1) * BL)
        in_maps.append(
            {
                "l3": np.ascontiguousarray(f["l3_points"][sl]),
                "x2": np.ascontiguousarray(f["x2_points"][sl]),
                "mf1": np.ascontiguousarray(
                    np.transpose(f["mem_f1"][sl], (1, 0, 2)).reshape(CM, ROWS)
                ),
                "mf2": np.ascontiguousarray(
                    np.transpose(f["mem_f2"][sl], (1, 0, 2)).reshape(CM, ROWS)
                ),
                "wp": wp,
            }
        )
    return _CACHE["nc"], in_maps


def _run(inputs, trace=False):
    nc, in_maps = _prep(inputs)
    res = run_bass_kernel_spmd(nc, in_maps, core_ids=list(range(NCORES)), trace=trace)
    out1 = np.concatenate([res.results[c]["out1"] for c in range(NCORES)], axis=0)
    out2 = np.concatenate([res.results[c]["out2"] for c in range(NCORES)], axis=0)
    return (out1, out2), res


def kernel(**inputs):
    (out1, out2), _ = _run(inputs, trace=bool(os.environ.get("KERNEL_TRACE")))
    return out1, out2
